# revision 40
# baseline (speedup 1.0000x reference)
"""Causal self-attention kernel for Trainium2, sharded over 8 NeuronCores.

Problem: x:(2048,2,768) f32, 12 heads, head_dim 64.
Sharding: batch (2) x head-groups (4 groups of 3 heads) -> 8 cores.
Each core computes q/k/v projections for its (batch, 3 heads), causal
flash-style attention, and a partial c_proj contribution. The host sums the
4 partial outputs per batch (the "all-reduce") and adds bo.

Device-side layout notes:
  - Matmul operands are bf16 (1 cycle/row on PE; fp32 accumulate in PSUM).
  - Scores are computed TRANSPOSED: scoresT[t, s] so softmax's denominator
    comes from a ones-column appended to V (m=65 matmul) and the exp runs
    along the free axis; no PE transposes of the probability matrix needed.
  - Heads h0,h1 share a 2-bank PSUM score tile (one merged exp); h2 has its
    own 1-bank tile. The attention loop is software-pipelined one tile deep:
    scores/exp for tile t+1 are emitted before attV of tile t, so the PE
    never waits on the scalar engine's exp. Background PE work (next
    block's projections, previous block's c_proj) is woven between the
    scores and attV of each tile.
    (fp8 DoubleRow scores were tried and reverted: numerically fine at
    7.7e-3 but no speedup on HW — DoubleRow ran at the same ns/row as
    bf16 while the required 96-wide projection passes added 15us.)
  - Causal masking: diagonal 128x512 score tiles restrict the live column
    range (lo); the remaining 128x128 triangular block is zeroed post-exp
    by a bf16 upper-triangular mask multiply on the vector engine.
  - q/k projection tails (dims 128:192 of the local 192) are computed by a
    single matmul per (k,block) with the two 64-wide weight tails packed
    side by side into a 128-wide stationary matrix.
"""

import os
import sys

sys.path.insert(0, "/opt/trn_rl_repo")

import numpy as np

import concourse.bass as bass  # noqa: F401  (import keeps bass registered)
import concourse.tile as tile
from concourse import bacc, bass_utils, library_config, mybir

F32 = mybir.dt.float32
BF16 = mybir.dt.bfloat16

S = 2048          # sequence length
B = 2             # batch
D = 768           # d_model
H = 12            # total heads
HD = 64           # head dim
NH = 3            # heads per core
DKL = NH * HD     # local q/k/v width = 192
KT = 6            # k-tiles over D (6 x 128)
WW = 128 * 3 + DKL  # packed weight width per k-tile: wq|wk|wqk|wv
SB = 512          # s-block width
QB = S // SB      # 4 q-blocks
TT = S // 128     # 16 t-tiles
SCALE = 1.0 / np.sqrt(HD)

_PROGRAM_CACHE = {}
LAST_EXEC_NS = None


def _build_program():
    nc = bacc.Bacc("TRN2", target_bir_lowering=False, debug=False, num_devices=8)

    xt_d = nc.dram_tensor("xt", [KT, 128, S], BF16, kind="ExternalInput").ap()
    w_d = nc.dram_tensor("wall", [KT, 128, WW], BF16, kind="ExternalInput").ap()
    wo_d = nc.dram_tensor("wo", [DKL, D], BF16, kind="ExternalInput").ap()
    tri_d = nc.dram_tensor("trimask", [128, 128], BF16, kind="ExternalInput").ap()
    out_d = nc.dram_tensor("outT", [D, S], BF16, kind="ExternalOutput").ap()

    EXP = mybir.ActivationFunctionType.Exp
    MUL = mybir.AluOpType.mult

    with tile.TileContext(nc) as tc:
        with (
            tc.tile_pool(name="xp", bufs=1) as xp,
            tc.tile_pool(name="wp", bufs=1) as wp,
            tc.tile_pool(name="qk", bufs=1) as qk,
            tc.tile_pool(name="vp", bufs=1) as vp,
            tc.tile_pool(name="ep", bufs=3) as ep,
            tc.tile_pool(name="ys", bufs=1) as ys,
            tc.tile_pool(name="dn", bufs=4) as dn,
            tc.tile_pool(name="op", bufs=2) as op,
            tc.tile_pool(name="psA", bufs=1, space="PSUM") as psA,
            tc.tile_pool(name="psA2", bufs=1, space="PSUM") as psA2,
            tc.tile_pool(name="psB", bufs=3, space="PSUM") as psB,
            tc.tile_pool(name="psC", bufs=2, space="PSUM") as psC,
        ):
            nc.gpsimd.load_library(library_config.attn)

            # ---- Phase 0: DMA inputs ----
            # Interleave x block-0 columns with the packed weights per k-tile
            # so the block-0 projections unblock after the first few
            # transfers; the x tail streams from the gpsimd queue.
            xts, ws = [], []
            for k in range(KT):
                xts.append(xp.tile([128, S], BF16, tag=f"x{k}", name=f"xt{k}"))
                ws.append(wp.tile([128, WW], BF16, tag=f"w{k}", name=f"wt{k}"))
            for k in range(KT):
                nc.sync.dma_start(xts[k][:, 0:SB], xt_d[k][:, 0:SB])
                nc.sync.dma_start(ws[k][:], w_d[k])
            # x tail split: block-1 columns first (feeds the background
            # projections woven into qb0), then the rest
            for k in range(KT):
                nc.scalar.dma_start(xts[k][:, SB : 2 * SB], xt_d[k][:, SB : 2 * SB])
            for k in range(KT):
                nc.scalar.dma_start(xts[k][:, 2 * SB : S], xt_d[k][:, 2 * SB : S])
            wqs = [w[:, 0:128] for w in ws]
            wks = [w[:, 128:256] for w in ws]
            wqks = [w[:, 256:384] for w in ws]
            wvs = [w[:, 384:WW] for w in ws]
            wo1 = wp.tile([128, D], BF16, tag="wo1")
            nc.sync.dma_start(wo1[:], wo_d[0:128])
            wo2 = wp.tile([64, D], BF16, tag="wo2")
            nc.sync.dma_start(wo2[:], wo_d[128:DKL])
            tri = wp.tile([128, 128], BF16, tag="tri")
            nc.sync.dma_start(tri[:], tri_d)

            qA = qk.tile([128, S], BF16, tag="qA")
            qB_ = qk.tile([64, S], BF16, tag="qB")
            kA = qk.tile([128, S], BF16, tag="kA")
            kB = qk.tile([64, S], BF16, tag="kB")
            yA = ys.tile([128, S], BF16, tag="yA")
            yB = ys.tile([64, S], BF16, tag="yB")

            vas = [None] * TT

            def qkv_groups(ncol):
                c0, c1 = ncol * SB, (ncol + 1) * SB

                def g_q(n=ncol):
                    ps = psC.tile([128, SB], F32, tag="mm", name=f"pq_{n}")
                    for k in range(KT):
                        nc.tensor.matmul(
                            ps[:], wqs[k], xts[k][:, c0:c1],
                            start=(k == 0), stop=(k == KT - 1),
                        )
                    nc.vector.tensor_copy(qA[:, c0:c1], ps[:])

                def g_k(n=ncol):
                    ps = psC.tile([128, SB], F32, tag="mm", name=f"pk_{n}")
                    for k in range(KT):
                        nc.tensor.matmul(
                            ps[:], wks[k], xts[k][:, c0:c1],
                            start=(k == 0), stop=(k == KT - 1),
                        )
                    nc.vector.tensor_copy(kA[:, c0:c1], ps[:])

                def g_tail(n=ncol):
                    # one matmul computes both 64-wide tails: out partitions
                    # 0:64 = q tail, 64:128 = k tail
                    ps = psC.tile([128, SB], F32, tag="mm", name=f"pt_{n}")
                    for k in range(KT):
                        nc.tensor.matmul(
                            ps[:], wqks[k], xts[k][:, c0:c1],
                            start=(k == 0), stop=(k == KT - 1),
                        )
                    nc.vector.tensor_copy(qB_[:, c0:c1], ps[0:64, :])
                    nc.vector.tensor_copy(kB[:, c0:c1], ps[64:128, :])

                gs = [g_q, g_k, g_tail]

                def mk_v(t):
                    def g_v():
                        ps = psC.tile([128, SB], F32, tag="mm", name=f"pv{t}")
                        for k in range(KT):
                            nc.tensor.matmul(
                                ps[:, 0:DKL],
                                xts[k][:, t * 128 : (t + 1) * 128],
                                wvs[k],
                                start=(k == 0), stop=(k == KT - 1),
                            )
                        va = vp.tile(
                            [128, NH * (HD + 1)], BF16, tag=f"v{t}", name=f"va{t}"
                        )
                        var = va[:].rearrange("p (h c) -> p h c", c=HD + 1)
                        nc.gpsimd.memset(var[:, :, HD : HD + 1], 1.0)
                        nc.vector.tensor_copy(
                            var[:, :, 0:HD],
                            ps[:, 0:DKL].rearrange("p (h d) -> p h d", d=HD),
                        )
                        vas[t] = va
                    return g_v

                return gs + [mk_v(t) for t in range(4 * ncol, 4 * ncol + 4)]

            def cproj_groups(qb):
                st = op.tile([128, 6 * SB], BF16, tag="st", name=f"st_{qb}")

                def mk(mc):
                    def g():
                        # wo2 (yB, head 2) first: its divide finishes first,
                        # shortening the tail chain after the last q-block.
                        ps = psC.tile([128, SB], F32, tag="mm", name=f"cp_{qb}_{mc}")
                        nc.tensor.matmul(
                            ps[:],
                            wo2[:, mc * 128 : (mc + 1) * 128],
                            yB[:, qb * SB : (qb + 1) * SB],
                            start=True, stop=False,
                        )
                        nc.tensor.matmul(
                            ps[:],
                            wo1[:, mc * 128 : (mc + 1) * 128],
                            yA[:, qb * SB : (qb + 1) * SB],
                            start=False, stop=True,
                        )
                        nc.scalar.copy(st[:, mc * SB : (mc + 1) * SB], ps[:])
                        # last q-block ships per-mc so the final transfer is
                        # small; earlier blocks ship 256-row pairs
                        if qb == QB - 1:
                            nc.sync.dma_start(
                                out_d[
                                    mc * 128 : (mc + 1) * 128,
                                    qb * SB : (qb + 1) * SB,
                                ],
                                st[:, mc * SB : (mc + 1) * SB],
                            )
                        elif mc % 2 == 1:
                            m0 = mc - 1
                            nc.sync.dma_start(
                                out_d[
                                    m0 * 128 : (mc + 1) * 128,
                                    qb * SB : (qb + 1) * SB,
                                ].rearrange("(m p) c -> p m c", p=128),
                                st[:, m0 * SB : (mc + 1) * SB].rearrange(
                                    "p (m c) -> p m c", c=SB
                                ),
                            )
                    return g
                return [mk(mc) for mc in range(D // 128)]

            def att_scores_exp(qb, t):
                """Scores for heads 0,1 into a 2-bank PSUM tile and head 2
                into a 1-bank tile; two merged exps; DVE triangular mask on
                diagonal tiles. Returns (ex01, ex2, lo)."""
                d = t * 128 - qb * SB
                lo = d if d >= 0 else 0
                sq = qA[:, qb * SB + lo : (qb + 1) * SB]
                sqB = qB_[:, qb * SB + lo : (qb + 1) * SB]
                tsl = slice(t * 128, (t + 1) * 128)
                sc01 = psA.tile([128, 2 * SB], F32, tag="sc01", name=f"sc01_{qb}_{t}")
                sc2 = psA2.tile([128, SB], F32, tag="sc2", name=f"sc2_{qb}_{t}")
                nc.tensor.matmul(
                    sc01[:, lo:SB], kA[0:64, tsl], sq[0:64, :], start=True, stop=True,
                )
                nc.tensor.matmul(
                    sc01[:, SB + lo : 2 * SB], kA[64:128, tsl], sq[64:128, :],
                    start=True, stop=True,
                )
                nc.tensor.matmul(
                    sc2[:, lo:SB], kB[0:64, tsl], sqB[0:64, :],
                    start=True, stop=True,
                )
                ex01 = ep.tile([128, 2 * SB], BF16, tag="ex01", name=f"ex01_{qb}_{t}")
                ex2 = ep.tile([128, SB], BF16, tag="ex2", name=f"ex2_{qb}_{t}")
                nc.scalar.activation(
                    ex01[:].rearrange("p (h c) -> p h c", c=SB)[:, :, lo:SB],
                    sc01[:].rearrange("p (h c) -> p h c", c=SB)[:, :, lo:SB],
                    EXP, scale=float(SCALE),
                )
                nc.scalar.activation(
                    ex2[:, lo:SB], sc2[:, lo:SB], EXP, scale=float(SCALE)
                )
                if d >= 0:
                    for h in range(2):
                        seg = ex01[:, h * SB + lo : h * SB + lo + 128]
                        nc.vector.tensor_tensor(seg, seg, tri[:], MUL)
                    seg = ex2[:, lo : lo + 128]
                    nc.vector.tensor_tensor(seg, seg, tri[:], MUL)
                return ex01, ex2, lo

            def att_v(qb, t, yps, nt, ex01, ex2, lo):
                for h in range(NH):
                    src = ex2[:, lo:SB] if h == 2 else ex01[:, h * SB + lo : (h + 1) * SB]
                    nc.tensor.matmul(
                        yps[h][:, lo:SB],
                        vas[t][:, h * (HD + 1) : (h + 1) * (HD + 1)],
                        src,
                        start=(t == 0), stop=(t == nt - 1),
                    )

            def divides(qb, yps):
                for h in (2, 0, 1):
                    dr = dn.tile([1, SB], F32, tag="dr", name=f"dr{qb}{h}")
                    nc.vector.tensor_copy(dr[:], yps[h][HD : HD + 1, :])
                    rc = dn.tile([1, SB], F32, tag="rc", name=f"rc{qb}{h}")
                    nc.vector.reciprocal_approx_fast(rc[:], dr[:])
                    bc = dn.tile([64, SB], F32, tag="bc", name=f"bc{qb}{h}")
                    nc.gpsimd.partition_broadcast(bc[:], rc[:], channels=64)
                    if h == 0:
                        dst = yA[0:64, qb * SB : (qb + 1) * SB]
                    elif h == 1:
                        dst = yA[64:128, qb * SB : (qb + 1) * SB]
                    else:
                        dst = yB[0:64, qb * SB : (qb + 1) * SB]
                    nc.vector.tensor_tensor(dst, yps[h][0:HD, :], bc[:], MUL)

            from collections import deque

            bg = deque()
            g0 = qkv_groups(0)
            for g in g0[:4]:  # q, k, tail, v0 — minimum to start tile 0
                g()
            bg.extend(g0[4:])  # v1..v3 woven into qb0's tiles
            for qb in range(QB):
                if qb + 1 < QB:
                    bg.extend(qkv_groups(qb + 1))
                if qb >= 1:
                    bg.extend(cproj_groups(qb - 1))
                nt = 4 * qb + 4
                yps = [
                    psB.tile([HD + 1, SB], F32, tag="ya", name=f"yps_{qb}_{h}")
                    for h in range(NH)
                ]
                nbg = len(bg)
                emitted = 0
                pend = None
                # On the last q-block, hold back a few bg groups to keep the
                # PE busy while the final divide chain runs on DVE/gpsimd.
                # keep ~one bg group in reserve: emitted right after the
                # divides so the PE has work across the q-block boundary
                pace = nt + 1
                for i in range(nt):
                    cur = att_scores_exp(qb, i)
                    want = (i + 1) * nbg // pace
                    while emitted < want and bg:
                        bg.popleft()()
                        emitted += 1
                    if pend is not None:
                        att_v(qb, i - 1, yps, nt, *pend)
                    pend = cur
                att_v(qb, nt - 1, yps, nt, *pend)
                divides(qb, yps)
                while bg:
                    bg.popleft()()
            for g in cproj_groups(QB - 1):
                g()

    nc.compile()
    return nc


def kernel(x, Wq, bq, Wk, bk, Wv, bv, Wo, bo):
    global LAST_EXEC_NS
    x = np.asarray(x, dtype=np.float32)
    Wq = np.asarray(Wq, dtype=np.float32)
    Wk = np.asarray(Wk, dtype=np.float32)
    Wv = np.asarray(Wv, dtype=np.float32)
    Wo = np.asarray(Wo, dtype=np.float32)
    bq = np.asarray(bq, dtype=np.float32)
    bk = np.asarray(bk, dtype=np.float32)
    bv = np.asarray(bv, dtype=np.float32)
    bo = np.asarray(bo, dtype=np.float32)

    # The device program folds no biases; nonzero bq/bk/bv (unused in this
    # problem) fall back to a host reference computation.
    if np.any(bq) or np.any(bk) or np.any(bv):
        q = (x @ Wq + bq).reshape(S, B, H, HD)
        k = (x @ Wk + bk).reshape(S, B, H, HD)
        v = (x @ Wv + bv).reshape(S, B, H, HD)
        att = np.einsum("sbhd,tbhd->bhst", q, k) * SCALE
        causal = np.triu(np.ones((S, S), dtype=bool), k=1)
        att = np.where(causal[None, None], -np.inf, att)
        att = att - att.max(axis=-1, keepdims=True)
        att = np.exp(att)
        att = att / att.sum(axis=-1, keepdims=True)
        y = np.einsum("bhst,tbhd->sbhd", att, v).reshape(S, B, D)
        return (y @ Wo + bo).astype(np.float32)

    if "prog" not in _PROGRAM_CACHE:
        _PROGRAM_CACHE["prog"] = _build_program()
    nc = _PROGRAM_CACHE["prog"]

    import ml_dtypes

    bf = ml_dtypes.bfloat16
    in_maps = []
    xT = [np.ascontiguousarray(x[:, b, :].T).astype(bf) for b in range(B)]
    trimask = np.triu(np.ones((128, 128))).astype(bf)
    for c in range(8):
        b, g = c // 4, c % 4
        sl = slice(g * DKL, (g + 1) * DKL)
        wq_g = Wq[:, sl]
        wk_g = Wk[:, sl]
        wall = np.concatenate(
            [wq_g[:, :128], wk_g[:, :128], wq_g[:, 128:], wk_g[:, 128:],
             Wv[:, sl]],
            axis=1,
        )  # [768, WW]
        in_maps.append({
            "xt": xT[b].reshape(KT, 128, S),
            "wall": np.ascontiguousarray(wall).astype(bf).reshape(KT, 128, WW),
            "wo": np.ascontiguousarray(Wo[sl, :]).astype(bf),
            "trimask": trimask,
        })

    trace = bool(int(os.environ.get("KERNEL_TRACE", "0")))
    res = bass_utils.run_bass_kernel_spmd(
        nc, in_maps, core_ids=list(range(8)), trace=trace
    )
    LAST_EXEC_NS = res.exec_time_ns

    out = np.zeros((S, B, D), dtype=np.float32)
    for c in range(8):
        b = c // 4
        out[:, b, :] += res.results[c]["outT"].astype(np.float32).T
    out += bo
    return out


# revision 43
# speedup vs baseline: 1.2343x; 1.2343x over previous
"""Causal self-attention kernel for Trainium2, sharded over 8 NeuronCores.

Problem: x:(2048,2,768) f32, 12 heads, head_dim 64.
Sharding: batch (2) x head-groups (4 groups of 3 heads) -> 8 cores.
Each core computes q/k/v projections for its (batch, 3 heads), causal
flash-style attention, and a partial c_proj contribution. The host sums the
4 partial outputs per batch (the "all-reduce") and adds bo.

Device-side layout notes:
  - Matmul operands are bf16 (1 cycle/row on PE; fp32 accumulate in PSUM).
  - Scores are computed TRANSPOSED: scoresT[t, s] so softmax's denominator
    comes from a ones-column appended to V (m=65 matmul) and the exp runs
    along the free axis; no PE transposes of the probability matrix needed.
  - Heads h0,h1 share a 2-bank PSUM score tile (one merged exp); h2 has its
    own 1-bank tile. The attention loop is software-pipelined one tile deep:
    scores/exp for tile t+1 are emitted before attV of tile t, so the PE
    never waits on the scalar engine's exp. Background PE work (next
    block's projections, previous block's c_proj) is woven between the
    scores and attV of each tile.
    (fp8 DoubleRow scores were tried and reverted: numerically fine at
    7.7e-3 but no speedup on HW — DoubleRow ran at the same ns/row as
    bf16 while the required 96-wide projection passes added 15us.)
  - Causal masking: diagonal 128x512 score tiles restrict the live column
    range (lo); the remaining 128x128 triangular block is zeroed post-exp
    by a bf16 upper-triangular mask multiply on the vector engine.
  - q/k projection tails (dims 128:192 of the local 192) are computed by a
    single matmul per (k,block) with the two 64-wide weight tails packed
    side by side into a 128-wide stationary matrix.
"""

import os
import sys

sys.path.insert(0, "/opt/trn_rl_repo")

import numpy as np

import concourse.bass as bass  # noqa: F401  (import keeps bass registered)
import concourse.tile as tile
from concourse import bacc, bass_utils, library_config, mybir

F32 = mybir.dt.float32
BF16 = mybir.dt.bfloat16

S = 2048          # sequence length
B = 2             # batch
D = 768           # d_model
H = 12            # total heads
HD = 64           # head dim
NH = 3            # heads per core
DKL = NH * HD     # local q/k/v width = 192
KT = 6            # k-tiles over D (6 x 128)
WW = 128 * 3 + DKL  # packed weight width per k-tile: wq|wk|wqk|wv
SB = 512          # s-block width
QB = S // SB      # 4 q-blocks
TT = S // 128     # 16 t-tiles
SCALE = 1.0 / np.sqrt(HD)

_PROGRAM_CACHE = {}
LAST_EXEC_NS = None


def _build_program():
    nc = bacc.Bacc("TRN2", target_bir_lowering=False, debug=False, num_devices=8)

    xt_d = nc.dram_tensor("xt", [KT, 128, S], BF16, kind="ExternalInput").ap()
    w_d = nc.dram_tensor("wall", [KT, 128, WW], BF16, kind="ExternalInput").ap()
    wo_d = nc.dram_tensor("wo", [DKL, D], BF16, kind="ExternalInput").ap()
    tri_d = nc.dram_tensor("trimask", [128, 128], BF16, kind="ExternalInput").ap()
    out_d = nc.dram_tensor("outT", [D, S], BF16, kind="ExternalOutput").ap()

    EXP = mybir.ActivationFunctionType.Exp
    MUL = mybir.AluOpType.mult

    with tile.TileContext(nc) as tc:
        with (
            tc.tile_pool(name="xp", bufs=1) as xp,
            tc.tile_pool(name="wp", bufs=1) as wp,
            tc.tile_pool(name="qk", bufs=1) as qk,
            tc.tile_pool(name="vp", bufs=1) as vp,
            tc.tile_pool(name="ep", bufs=3) as ep,
            tc.tile_pool(name="ys", bufs=1) as ys,
            tc.tile_pool(name="dn", bufs=4) as dn,
            tc.tile_pool(name="op", bufs=2) as op,
            tc.tile_pool(name="psA", bufs=1, space="PSUM") as psA,
            tc.tile_pool(name="psA2", bufs=1, space="PSUM") as psA2,
            tc.tile_pool(name="psB", bufs=3, space="PSUM") as psB,
            tc.tile_pool(name="psC", bufs=2, space="PSUM") as psC,
        ):
            nc.gpsimd.load_library(library_config.attn)

            # ---- Phase 0: DMA inputs ----
            # Interleave x block-0 columns with the packed weights per k-tile
            # so the block-0 projections unblock after the first few
            # transfers; the x tail streams from the scalar (ACT) DMA queue.
            xts, ws = [], []
            for k in range(KT):
                xts.append(xp.tile([128, S], BF16, tag=f"x{k}", name=f"xt{k}"))
                ws.append(wp.tile([128, WW], BF16, tag=f"w{k}", name=f"wt{k}"))
            for k in range(KT):
                nc.sync.dma_start(xts[k][:, 0:SB], xt_d[k][:, 0:SB])
                nc.sync.dma_start(ws[k][:], w_d[k])
            # x tail split: block-1 columns first (feeds the background
            # projections woven into qb0), then the rest
            for k in range(KT):
                nc.scalar.dma_start(xts[k][:, SB : 2 * SB], xt_d[k][:, SB : 2 * SB])
            for k in range(KT):
                nc.scalar.dma_start(xts[k][:, 2 * SB : S], xt_d[k][:, 2 * SB : S])
            wqs = [w[:, 0:128] for w in ws]
            wks = [w[:, 128:256] for w in ws]
            wqks = [w[:, 256:384] for w in ws]
            wvs = [w[:, 384:WW] for w in ws]
            wo1 = wp.tile([128, D], BF16, tag="wo1")
            nc.sync.dma_start(wo1[:], wo_d[0:128])
            wo2 = wp.tile([64, D], BF16, tag="wo2")
            nc.sync.dma_start(wo2[:], wo_d[128:DKL])
            tri = wp.tile([128, 128], BF16, tag="tri")
            nc.sync.dma_start(tri[:], tri_d)

            qA = qk.tile([128, S], BF16, tag="qA")
            qB_ = qk.tile([64, S], BF16, tag="qB")
            kA = qk.tile([128, S], BF16, tag="kA")
            kB = qk.tile([64, S], BF16, tag="kB")
            yA = ys.tile([128, S], BF16, tag="yA")
            yB = ys.tile([64, S], BF16, tag="yB")

            vas = [None] * TT

            def qkv_groups(ncol):
                c0, c1 = ncol * SB, (ncol + 1) * SB

                def g_q(n=ncol):
                    ps = psC.tile([128, SB], F32, tag="mm", name=f"pq_{n}")
                    for k in range(KT):
                        nc.tensor.matmul(
                            ps[:], wqs[k], xts[k][:, c0:c1],
                            start=(k == 0), stop=(k == KT - 1),
                        )
                    nc.vector.tensor_copy(qA[:, c0:c1], ps[:])

                def g_k(n=ncol):
                    ps = psC.tile([128, SB], F32, tag="mm", name=f"pk_{n}")
                    for k in range(KT):
                        nc.tensor.matmul(
                            ps[:], wks[k], xts[k][:, c0:c1],
                            start=(k == 0), stop=(k == KT - 1),
                        )
                    nc.vector.tensor_copy(kA[:, c0:c1], ps[:])

                def g_tail(n=ncol):
                    # one matmul computes both 64-wide tails: out partitions
                    # 0:64 = q tail, 64:128 = k tail
                    ps = psC.tile([128, SB], F32, tag="mm", name=f"pt_{n}")
                    for k in range(KT):
                        nc.tensor.matmul(
                            ps[:], wqks[k], xts[k][:, c0:c1],
                            start=(k == 0), stop=(k == KT - 1),
                        )
                    nc.vector.tensor_copy(qB_[:, c0:c1], ps[0:64, :])
                    nc.vector.tensor_copy(kB[:, c0:c1], ps[64:128, :])

                gs = [g_q, g_k, g_tail]

                def mk_v(t):
                    def g_v():
                        ps = psC.tile([128, SB], F32, tag="mm", name=f"pv{t}")
                        for k in range(KT):
                            nc.tensor.matmul(
                                ps[:, 0:DKL],
                                xts[k][:, t * 128 : (t + 1) * 128],
                                wvs[k],
                                start=(k == 0), stop=(k == KT - 1),
                            )
                        va = vp.tile(
                            [128, NH * (HD + 1)], BF16, tag=f"v{t}", name=f"va{t}"
                        )
                        var = va[:].rearrange("p (h c) -> p h c", c=HD + 1)
                        nc.gpsimd.memset(var[:, :, HD : HD + 1], 1.0)
                        nc.vector.tensor_copy(
                            var[:, :, 0:HD],
                            ps[:, 0:DKL].rearrange("p (h d) -> p h d", d=HD),
                        )
                        vas[t] = va
                    return g_v

                return gs + [mk_v(t) for t in range(4 * ncol, 4 * ncol + 4)]

            def cproj_groups(qb):
                st = op.tile([128, 6 * SB], BF16, tag="st", name=f"st_{qb}")

                def mk(mc):
                    def g():
                        # wo2 (yB, head 2) first: its divide finishes first,
                        # shortening the tail chain after the last q-block.
                        ps = psC.tile([128, SB], F32, tag="mm", name=f"cp_{qb}_{mc}")
                        nc.tensor.matmul(
                            ps[:],
                            wo2[:, mc * 128 : (mc + 1) * 128],
                            yB[:, qb * SB : (qb + 1) * SB],
                            start=True, stop=False,
                        )
                        nc.tensor.matmul(
                            ps[:],
                            wo1[:, mc * 128 : (mc + 1) * 128],
                            yA[:, qb * SB : (qb + 1) * SB],
                            start=False, stop=True,
                        )
                        nc.vector.tensor_copy(st[:, mc * SB : (mc + 1) * SB], ps[:])
                        # last q-block ships per-mc so the final transfer is
                        # small; earlier blocks ship 256-row pairs
                        if qb == QB - 1:
                            nc.sync.dma_start(
                                out_d[
                                    mc * 128 : (mc + 1) * 128,
                                    qb * SB : (qb + 1) * SB,
                                ],
                                st[:, mc * SB : (mc + 1) * SB],
                            )
                        elif mc % 2 == 1:
                            m0 = mc - 1
                            nc.sync.dma_start(
                                out_d[
                                    m0 * 128 : (mc + 1) * 128,
                                    qb * SB : (qb + 1) * SB,
                                ].rearrange("(m p) c -> p m c", p=128),
                                st[:, m0 * SB : (mc + 1) * SB].rearrange(
                                    "p (m c) -> p m c", c=SB
                                ),
                            )
                    return g
                return [mk(mc) for mc in range(D // 128)]

            def att_scores_exp(qb, t):
                """Scores for heads 0,1 into a 2-bank PSUM tile and head 2
                into a 1-bank tile; two merged exps; DVE triangular mask on
                diagonal tiles. Returns (ex01, ex2, lo)."""
                d = t * 128 - qb * SB
                lo = d if d >= 0 else 0
                sq = qA[:, qb * SB + lo : (qb + 1) * SB]
                sqB = qB_[:, qb * SB + lo : (qb + 1) * SB]
                tsl = slice(t * 128, (t + 1) * 128)
                sc01 = psA.tile([128, 2 * SB], F32, tag="sc01", name=f"sc01_{qb}_{t}")
                sc2 = psA2.tile([128, SB], F32, tag="sc2", name=f"sc2_{qb}_{t}")
                nc.tensor.matmul(
                    sc01[:, lo:SB], kA[0:64, tsl], sq[0:64, :], start=True, stop=True,
                )
                nc.tensor.matmul(
                    sc01[:, SB + lo : 2 * SB], kA[64:128, tsl], sq[64:128, :],
                    start=True, stop=True,
                )
                nc.tensor.matmul(
                    sc2[:, lo:SB], kB[0:64, tsl], sqB[0:64, :],
                    start=True, stop=True,
                )
                ex01 = ep.tile([128, 2 * SB], BF16, tag="ex01", name=f"ex01_{qb}_{t}")
                ex2 = ep.tile([128, SB], BF16, tag="ex2", name=f"ex2_{qb}_{t}")
                nc.scalar.activation(
                    ex01[:].rearrange("p (h c) -> p h c", c=SB)[:, :, lo:SB],
                    sc01[:].rearrange("p (h c) -> p h c", c=SB)[:, :, lo:SB],
                    EXP, scale=float(SCALE),
                )
                nc.scalar.activation(
                    ex2[:, lo:SB], sc2[:, lo:SB], EXP, scale=float(SCALE)
                )
                if d >= 0:
                    for h in range(2):
                        seg = ex01[:, h * SB + lo : h * SB + lo + 128]
                        nc.vector.tensor_tensor(seg, seg, tri[:], MUL)
                    seg = ex2[:, lo : lo + 128]
                    nc.vector.tensor_tensor(seg, seg, tri[:], MUL)
                return ex01, ex2, lo

            def att_v(qb, t, yps, nt, ex01, ex2, lo):
                for h in range(NH):
                    src = ex2[:, lo:SB] if h == 2 else ex01[:, h * SB + lo : (h + 1) * SB]
                    nc.tensor.matmul(
                        yps[h][:, lo:SB],
                        vas[t][:, h * (HD + 1) : (h + 1) * (HD + 1)],
                        src,
                        start=(t == 0), stop=(t == nt - 1),
                    )

            def divides(qb, yps):
                for h in (2, 0, 1):
                    dr = dn.tile([1, SB], F32, tag="dr", name=f"dr{qb}{h}")
                    nc.vector.tensor_copy(dr[:], yps[h][HD : HD + 1, :])
                    rc = dn.tile([1, SB], F32, tag="rc", name=f"rc{qb}{h}")
                    nc.vector.reciprocal_approx_fast(rc[:], dr[:])
                    bc = dn.tile([64, SB], F32, tag="bc", name=f"bc{qb}{h}")
                    nc.gpsimd.partition_broadcast(bc[:], rc[:], channels=64)
                    if h == 0:
                        dst = yA[0:64, qb * SB : (qb + 1) * SB]
                    elif h == 1:
                        dst = yA[64:128, qb * SB : (qb + 1) * SB]
                    else:
                        dst = yB[0:64, qb * SB : (qb + 1) * SB]
                    nc.vector.tensor_tensor(dst, yps[h][0:HD, :], bc[:], MUL)

            from collections import deque

            bg = deque()
            g0 = qkv_groups(0)
            for g in g0[:4]:  # q, k, tail, v0 — minimum to start tile 0
                g()
            bg.extend(g0[4:])  # v1..v3 woven into qb0's tiles
            for qb in range(QB):
                if qb + 1 < QB:
                    bg.extend(qkv_groups(qb + 1))
                if qb >= 1:
                    bg.extend(cproj_groups(qb - 1))
                nt = 4 * qb + 4
                yps = [
                    psB.tile([HD + 1, SB], F32, tag="ya", name=f"yps_{qb}_{h}")
                    for h in range(NH)
                ]
                nbg = len(bg)
                emitted = 0
                pend = None
                # On the last q-block, hold back a few bg groups to keep the
                # PE busy while the final divide chain runs on DVE/gpsimd.
                # keep ~one bg group in reserve: emitted right after the
                # divides so the PE has work across the q-block boundary
                pace = nt
                for i in range(nt):
                    cur = att_scores_exp(qb, i)
                    want = (i + 1) * nbg // pace
                    while emitted < want and bg:
                        bg.popleft()()
                        emitted += 1
                    if pend is not None:
                        att_v(qb, i - 1, yps, nt, *pend)
                    pend = cur
                att_v(qb, nt - 1, yps, nt, *pend)
                divides(qb, yps)
                while bg:
                    bg.popleft()()
            for g in cproj_groups(QB - 1):
                g()

    nc.compile()
    return nc


def kernel(x, Wq, bq, Wk, bk, Wv, bv, Wo, bo):
    global LAST_EXEC_NS
    x = np.asarray(x, dtype=np.float32)
    Wq = np.asarray(Wq, dtype=np.float32)
    Wk = np.asarray(Wk, dtype=np.float32)
    Wv = np.asarray(Wv, dtype=np.float32)
    Wo = np.asarray(Wo, dtype=np.float32)
    bq = np.asarray(bq, dtype=np.float32)
    bk = np.asarray(bk, dtype=np.float32)
    bv = np.asarray(bv, dtype=np.float32)
    bo = np.asarray(bo, dtype=np.float32)

    # The device program folds no biases; nonzero bq/bk/bv (unused in this
    # problem) fall back to a host reference computation.
    if np.any(bq) or np.any(bk) or np.any(bv):
        q = (x @ Wq + bq).reshape(S, B, H, HD)
        k = (x @ Wk + bk).reshape(S, B, H, HD)
        v = (x @ Wv + bv).reshape(S, B, H, HD)
        att = np.einsum("sbhd,tbhd->bhst", q, k) * SCALE
        causal = np.triu(np.ones((S, S), dtype=bool), k=1)
        att = np.where(causal[None, None], -np.inf, att)
        att = att - att.max(axis=-1, keepdims=True)
        att = np.exp(att)
        att = att / att.sum(axis=-1, keepdims=True)
        y = np.einsum("bhst,tbhd->sbhd", att, v).reshape(S, B, D)
        return (y @ Wo + bo).astype(np.float32)

    if "prog" not in _PROGRAM_CACHE:
        _PROGRAM_CACHE["prog"] = _build_program()
    nc = _PROGRAM_CACHE["prog"]

    import ml_dtypes

    bf = ml_dtypes.bfloat16
    in_maps = []
    xT = [np.ascontiguousarray(x[:, b, :].T).astype(bf) for b in range(B)]
    trimask = np.triu(np.ones((128, 128))).astype(bf)
    for c in range(8):
        b, g = c // 4, c % 4
        sl = slice(g * DKL, (g + 1) * DKL)
        wq_g = Wq[:, sl]
        wk_g = Wk[:, sl]
        wall = np.concatenate(
            [wq_g[:, :128], wk_g[:, :128], wq_g[:, 128:], wk_g[:, 128:],
             Wv[:, sl]],
            axis=1,
        )  # [768, WW]
        in_maps.append({
            "xt": xT[b].reshape(KT, 128, S),
            "wall": np.ascontiguousarray(wall).astype(bf).reshape(KT, 128, WW),
            "wo": np.ascontiguousarray(Wo[sl, :]).astype(bf),
            "trimask": trimask,
        })

    trace = bool(int(os.environ.get("KERNEL_TRACE", "0")))
    res = bass_utils.run_bass_kernel_spmd(
        nc, in_maps, core_ids=list(range(8)), trace=trace
    )
    LAST_EXEC_NS = res.exec_time_ns

    out = np.zeros((S, B, D), dtype=np.float32)
    for c in range(8):
        b = c // 4
        out[:, b, :] += res.results[c]["outT"].astype(np.float32).T
    out += bo
    return out
